# revision 38
# baseline (speedup 1.0000x reference)
"""Fused attention kernel for Trainium2, 8 NeuronCores.

Problem: B=4, T=2048, C=1024, nh=16, hs=64, fused QKV (chunk order k,q,v),
softmax attention, then (faithful reference bug) reshape (B,nh,T,hs)->(B,T,C)
directly before the output projection.

Key structural fact: with the buggy reshape, head h's attention output
occupies exactly rows [h*128, (h+1)*128) of the reshaped (T, C) matrix
(row tau = h*128 + t//16, col = (t%16)*64 + d). So everything after the
QKV projection is fully independent per (batch, head) pair; the output
projection needs no cross-head reduction.

Sharding: 8 cores = 4 batches x 2 head-groups (8 heads each). Each core
computes its batch's QKV slice and its 8 heads end-to-end. No collectives.

v2: single fused pipeline. The whole kernel is one stream of 256
attention iterations (hp, ic, j) with the QKV projection folded in as
prefix (K0,Q0,V) + per-block filler bursts, so the tensor engine never
drains and the scalar engine (exp) starts ~25us in. Scores operands in
bf16 (fp32 rhs streams ~1.5x slower). A fraction of the exp tiles run
on the DVE via a Schraudolph int16 bit-trick (exp error is a ~2% sawtooth
whose constant factor cancels in softmax normalization), keeping the
scalar engine below the tensor engine's per-iteration cost.
"""

import math
import sys

import numpy as np

sys.path.insert(0, "/opt/trn_rl_repo")

import ml_dtypes  # noqa: E402

B, T, C = 4, 2048, 1024
NH, HS = 16, 64
NCORES = 8
HPC = 8  # heads per core

_CACHE = {}

# Schraudolph fast-exp constants (bf16 bit pattern via int16):
# exp(0.125*x) ~= bitcast_bf16(int16(x * (0.125*128/ln2) + B)). B is shifted
# below 127*128=16256 to zero the mean of the sawtooth approximation error:
# a nonzero mean is a systematic softmax-weight bias for the key-blocks that
# take this path (the constant factor only cancels when ALL blocks share it).
EXP_A = 0.125 * 128.0 / math.log(2.0)
EXP_B = 16249.8


def _build():
    from contextlib import ExitStack

    import concourse.bass as bass  # noqa: F401
    import concourse.mybir as mybir
    from concourse import bacc, tile

    F32 = mybir.dt.float32
    BF16 = mybir.dt.bfloat16
    I16 = mybir.dt.int16
    ADD = mybir.AluOpType.add
    MULT = mybir.AluOpType.mult
    EXP = mybir.ActivationFunctionType.Exp
    IDENT = mybir.ActivationFunctionType.Identity

    nc = bacc.Bacc()
    # DRAM inputs (host-prepacked for contiguous, full-rate DMA rows)
    xq = nc.dram_tensor("xq", [8, 128, 2048], BF16, kind="ExternalInput")
    wkq = nc.dram_tensor("wkq", [8, 128, 1024], BF16, kind="ExternalInput")
    wvd = nc.dram_tensor("wvd", [128, 4096], BF16, kind="ExternalInput")
    bkq = nc.dram_tensor("bkq", [128, 8], F32, kind="ExternalInput")
    bv = nc.dram_tensor("bv", [128, 512], F32, kind="ExternalInput")
    wp = nc.dram_tensor("wp", [64, 16, 1024], BF16, kind="ExternalInput")
    pb = nc.dram_tensor("pb", [128, 1024], F32, kind="ExternalInput")
    vones = nc.dram_tensor("vones", [128, 1], BF16, kind="ExternalInput")
    onesr = nc.dram_tensor("onesr", [1, 64], F32, kind="ExternalInput")
    y = nc.dram_tensor("y", [128, 8, 1024], F32, kind="ExternalOutput")

    with tile.TileContext(nc) as tc, ExitStack() as ctx:
        persist = ctx.enter_context(tc.tile_pool(name="persist", bufs=1))
        utp = ctx.enter_context(tc.tile_pool(name="utp", bufs=10))
        usp = ctx.enter_context(tc.tile_pool(name="usp", bufs=2))
        nrm = ctx.enter_context(tc.tile_pool(name="nrm", bufs=2))
        yps = ctx.enter_context(tc.tile_pool(name="ysb", bufs=2))
        spx = ctx.enter_context(tc.tile_pool(name="spool", bufs=2, space="PSUM"))
        ypx = ctx.enter_context(tc.tile_pool(name="ypool", bufs=2, space="PSUM"))
        opx = ctx.enter_context(tc.tile_pool(name="opool", bufs=1, space="PSUM"))
        rpx = ctx.enter_context(tc.tile_pool(name="rpool", bufs=1, space="PSUM"))
        dpool = ctx.enter_context(tc.tile_pool(name="dpool", bufs=2, space="DRAM"))

        # ---- persistent tiles + priority-ordered input DMAs ----
        wkq_sb = [persist.tile([128, 1024], BF16, tag=f"wkq{mt}", name=f"wkq{mt}")
                  for mt in range(8)]
        xts = persist.tile([128, 8, 2048], BF16, tag="xts")
        wv_sb = persist.tile([128, 8, 512], BF16, tag="wv")
        bkq_sb = persist.tile([128, 8], F32, tag="bkq")
        bv_sb = persist.tile([128, 512], F32, tag="bv")
        vones_sb = persist.tile([128, 1], BF16, tag="vones")
        wp_sb = persist.tile([128, 16, 1024], BF16, tag="wp")
        pb_sb = persist.tile([128, 1024], F32, tag="pb")
        vbuf = persist.tile([128, 16, HPC, 64], BF16, tag="vbuf")
        qk = [persist.tile([128, 2048], BF16, tag=f"qk{mt}", name=f"qk{mt}")
              for mt in range(8)]
        ots = [persist.tile([128, 2048], BF16, tag=f"ot{hp}", name=f"ot{hp}")
               for hp in range(4)]

        # first-needed first: K0 weights, x half-quarters, Q0/K1/Q1 weights,
        # V weights, remaining KQ weights, then attention-phase constants.
        nc.sync.dma_start(wkq_sb[0], wkq[0])
        nc.sync.dma_start(bkq_sb, bkq[:])
        nc.sync.dma_start(
            xts[:, :, 0:256], xq[0].rearrange("p (a b) -> p a b", b=256))
        nc.sync.dma_start(
            xts[:, :, 256:512], xq[1].rearrange("p (a b) -> p a b", b=256))
        nc.sync.dma_start(wkq_sb[4], wkq[4])
        for q in range(2, 8):
            nc.sync.dma_start(
                xts[:, :, q * 256:(q + 1) * 256],
                xq[q].rearrange("p (a b) -> p a b", b=256))
        nc.sync.dma_start(wkq_sb[1], wkq[1])
        nc.sync.dma_start(wkq_sb[5], wkq[5])
        nc.sync.dma_start(wv_sb, wvd.rearrange("p (a b) -> p a b", b=512))
        nc.sync.dma_start(bv_sb, bv[:])
        for mt in (2, 6, 3, 7):
            nc.sync.dma_start(wkq_sb[mt], wkq[mt])
        nc.sync.dma_start(vones_sb, vones[:])
        onesr_sb = persist.tile([1, 64], F32, tag="onesr")
        nc.sync.dma_start(onesr_sb, onesr[:])
        nc.sync.dma_start(wp_sb[0:64], wp[:])
        nc.sync.dma_start(wp_sb[64:128], wp[:])
        nc.sync.dma_start(pb_sb, pb[:])

        # ---------------- emission helpers ----------------
        ITEMS = [(hp, ic, j) for hp in range(4) for ic in range(4)
                 for j in range(16)]
        DVE_J = {3, 7, 11, 15}
        uts = {}
        usums = {}

        def kq_group(mt, ic2, chunk=512):
            ps = spx.tile([128, 1024], F32, tag="sp", name=f"kq{mt}_{ic2}")
            for half in range(1024 // chunk):
                t0 = ic2 * 1024 + half * chunk
                for ct in range(8):
                    nc.tensor.matmul(
                        ps[:, half * chunk:(half + 1) * chunk],
                        wkq_sb[mt][:, ct * 128:(ct + 1) * 128],
                        xts[:, ct, t0:t0 + chunk],
                        start=(ct == 0), stop=(ct == 7))
            # Identity shares the exp_and_others act table: no table thrash
            nc.scalar.activation(
                qk[mt][:, ic2 * 1024:(ic2 + 1) * 1024], ps, IDENT,
                bias=bkq_sb[:, mt:mt + 1])

        def v_group(tt):
            ps = ypx.tile([128, 512], F32, tag="yp", name=f"v{tt}")
            for ct in range(8):
                nc.tensor.matmul(
                    ps, xts[:, ct, tt * 128:(tt + 1) * 128], wv_sb[:, ct, :],
                    start=(ct == 0), stop=(ct == 7))
            nc.vector.tensor_tensor(
                vbuf[:, tt, :, :],
                ps.rearrange("p (h d) -> p h d", d=64),
                bv_sb.rearrange("p (h d) -> p h d", d=64), ADD)

        def emit_score(idx):
            hp, ic, j = ITEMS[idx]
            kt = qk[hp]
            qt = qk[4 + hp]
            jsl = slice(j * 128, (j + 1) * 128)
            isl = slice(ic * 512, (ic + 1) * 512)
            sp = spx.tile([128, 1024], F32, tag="sp", name="sp")
            nc.tensor.matmul(sp[:, 0:512], kt[0:64, jsl], qt[0:64, isl],
                             start=True, stop=True)
            nc.tensor.matmul(sp[:, 512:1024], kt[64:128, jsl],
                             qt[64:128, isl], start=True, stop=True)
            ut = utp.tile([128, 1024], BF16, tag="ut", name="ut")
            if j in DVE_J:
                nc.vector.tensor_scalar(
                    ut.bitcast(I16), sp, EXP_A, EXP_B, MULT, ADD)
            else:
                nc.scalar.activation(ut, sp, EXP, scale=0.125)
            uts[idx] = ut
            # pairwise-tree partial sums of exp tiles on the DVE: one
            # denominator matmul pair per 4 key-tiles instead of per 1.
            if j % 4 == 1:
                # s1 on the otherwise-idle GPSIMD (slow but off both
                # critical engines; consumed ~4 groups later)
                s1 = usp.tile([128, 1024], BF16, tag="us1", name="us1")
                nc.gpsimd.tensor_tensor(s1, uts[idx - 1], ut, ADD)
                usums[(idx - 1) // 4] = s1
            elif j % 4 == 3:
                s2 = usp.tile([128, 1024], BF16, tag="us1", name="us2")
                nc.vector.tensor_tensor(s2, uts[idx - 1], ut, ADD)
                s12 = usp.tile([128, 1024], BF16, tag="us12", name="us12")
                nc.vector.tensor_tensor(s12, usums[(idx - 3) // 4], s2, ADD)
                usums[(idx - 3) // 4] = s12

        av_state = {}

        def emit_av(idx):
            hp, ic, j = ITEMS[idx]
            hA, hB = 2 * hp, 2 * hp + 1
            if j == 0:
                av_state["op"] = opx.tile([128, 512], F32, tag="op", name="op")
                av_state["rs"] = rpx.tile([33, 512], F32, tag="rs", name="rs")
            optile = av_state["op"]
            ut = uts.pop(idx)
            nc.tensor.matmul(optile[0:64, :], vbuf[:, j, hA, :], ut[:, 0:512],
                             start=(j == 0), stop=(j == 15))
            nc.tensor.matmul(optile[64:128, :], vbuf[:, j, hB, :],
                             ut[:, 512:1024],
                             start=(j == 0), stop=(j == 15),
                             tile_position=(0, 64))

        def emit_ones_quad(idx):
            # idx = last item of a quad; sums exp over key-tiles 4qd..4qd+3
            qd = (idx % 16) // 4
            rsps = av_state["rs"]
            us = usums.pop(idx // 4)
            nc.tensor.matmul(rsps[0:1, :], vones_sb, us[:, 0:512],
                             start=(qd == 0), stop=(qd == 3))
            nc.tensor.matmul(rsps[32:33, :], vones_sb, us[:, 512:1024],
                             start=(qd == 0), stop=(qd == 3),
                             tile_position=(0, 32))

        norm_state = {}

        def norm_stage1a(hp, ic):
            # osb copy gates the op-psum slot for the next block's AV j=0:
            # emit it first, straight after the block's last AV matmul.
            osb = nrm.tile([128, 512], F32, tag="osb", name="osb")
            nc.vector.tensor_copy(osb, av_state["op"])
            return osb

        def norm_stage1(hp, ic, osb):
            rsps = av_state["rs"]
            rsb = nrm.tile([33, 512], F32, tag="rsb", name="rsb")
            nc.vector.tensor_copy(rsb, rsps)
            scr1 = dpool.tile([1024], F32, tag="scr1", name="scr1")
            nc.sync.dma_start(
                scr1.rearrange("(r f) -> r f", r=2), rsb[0:33:32, :])
            rst = nrm.tile([128, 8], F32, tag="rst", name="rst")
            nc.sync.dma_start(rst, scr1.rearrange("(p f) -> p f", f=8))
            norm_state[(hp, ic)] = (osb, rst)

        def norm_tail_fast(hp, ic):
            # last block: skip the DRAM broadcast round-trip; reciprocal on
            # the row copies, partition-broadcast via a PE outer product into
            # now-free PSUM, multiply from there.
            optile, rsps = av_state["op"], av_state["rs"]
            rsb = nrm.tile([33, 512], F32, tag="rsb", name="rsb")
            nc.vector.tensor_copy(rsb, rsps)
            osb = nrm.tile([128, 512], F32, tag="osb", name="osb")
            nc.vector.tensor_copy(osb, optile)
            rsbB = persist.tile([1, 512], F32, tag="rsbB", name="rsbB")
            nc.sync.dma_start(rsbB, rsb[32:33, :])
            nc.vector.reciprocal(rsb[0:1, :], rsb[0:1, :])
            nc.vector.reciprocal(rsbB, rsbB)
            bc = spx.tile([128, 1024], F32, tag="sp", name="bc")
            nc.tensor.matmul(bc[0:64, 0:512], onesr_sb, rsb[0:1, :],
                             start=True, stop=True)
            nc.tensor.matmul(bc[64:128, 0:512], onesr_sb, rsbB,
                             start=True, stop=True, tile_position=(0, 64))
            nc.vector.tensor_tensor(
                ots[hp][:, ic * 512:(ic + 1) * 512], osb, bc[:, 0:512], MULT)

        def norm_stage2(hp, ic):
            osb, rst = norm_state[(hp, ic)]
            nc.vector.reciprocal(rst, rst)
            scr2 = dpool.tile([1024], F32, tag="scr2", name="scr2")
            nc.sync.dma_start(scr2.rearrange("(p f) -> p f", f=8), rst)
            bcsb = nrm.tile([128, 512], F32, tag="bcsb", name="bcsb")
            nc.sync.dma_start(
                bcsb[0:64, :], scr2[None, 0:512].to_broadcast((64, 512)))
            nc.sync.dma_start(
                bcsb[64:128, :], scr2[None, 512:1024].to_broadcast((64, 512)))
            norm_state[(hp, ic)] = (osb, bcsb)

        def norm_stage3(hp, ic):
            osb, bcsb = norm_state.pop((hp, ic))
            nc.vector.tensor_tensor(
                ots[hp][:, ic * 512:(ic + 1) * 512], osb, bcsb, MULT)

        proj_state = {}

        def proj_step(hp, q2, u):
            if "ypA" not in proj_state:
                proj_state["ypA"] = ypx.tile([128, 512], F32, tag="yp",
                                             name="ypA")
                proj_state["ypB"] = ypx.tile([128, 512], F32, tag="yp",
                                             name="ypB")
            ypA, ypB = proj_state["ypA"], proj_state["ypB"]
            otr = ots[hp].rearrange("d (t u) -> d u t", u=16)
            csl = slice(q2 * 512, (q2 + 1) * 512)
            nc.tensor.matmul(ypA, otr[0:64, u, :], wp_sb[0:64, u, csl],
                             start=(u == 0), stop=(u == 15))
            nc.tensor.matmul(ypB, otr[64:128, u, :], wp_sb[64:128, u, csl],
                             start=(u == 0), stop=(u == 15))
            if u == 15:
                for h, yp in ((2 * hp, ypA), (2 * hp + 1, ypB)):
                    ysb = yps.tile([128, 512], F32, tag="ysb", name="ysb")
                    nc.vector.tensor_tensor(ysb, yp, pb_sb[:, csl], ADD)
                    nc.sync.dma_start(y[:, h, csl], ysb)
                proj_state.clear()

        # ---------------- prefix: K0,Q0,K1,Q1, V, first scores -----------
        LOOK = 8
        nscore = 0

        def next_score():
            nonlocal nscore
            emit_score(nscore)
            nscore += 1

        kq_group(0, 0, chunk=256)
        kq_group(4, 0)
        kq_group(0, 1)
        kq_group(4, 1)
        kq_group(1, 0)
        next_score()
        kq_group(5, 0)
        next_score()
        kq_group(1, 1)
        next_score()
        kq_group(5, 1)
        next_score()
        for tt in range(16):
            v_group(tt)
            if tt in (5, 8, 11, 14) and nscore < LOOK:
                next_score()

        # ------------- main loop: 16 blocks x 8 groups of 2 iters --------
        FILLER = [(2, 0), (2, 1), (6, 0), (6, 1), (3, 0), (3, 1),
                  (7, 0), (7, 1)]
        BLOCKS = [(hp, ic) for hp in range(4) for ic in range(4)]
        proj_q = []
        prev_block = None

        for bi, (hp, ic) in enumerate(BLOCKS):
            if bi < 8:
                kq_group(*FILLER[bi])
            for g in range(8):
                w = 16 * bi + 2 * g
                if proj_q:
                    proj_step(*proj_q.pop(0))
                    if len(proj_q) > 8:
                        proj_step(*proj_q.pop(0))
                if nscore < 256:
                    next_score()
                if nscore < 256:
                    next_score()
                emit_av(w)
                emit_av(w + 1)
                if g == 7 and bi < 15:
                    osb = norm_stage1a(hp, ic)
                if g % 2 == 1:
                    emit_ones_quad(w + 1)
                if prev_block is not None:
                    if g == 2:
                        norm_stage2(*prev_block)
                    elif g == 4:
                        norm_stage3(*prev_block)
                    elif g == 5 and prev_block[1] == 3:
                        php = prev_block[0]
                        proj_q.extend([(php, q2, u) for q2 in range(2)
                                       for u in range(16)])
            if bi < 15:
                norm_stage1(hp, ic, osb)
            else:
                norm_tail_fast(hp, ic)
            prev_block = (hp, ic)

        # ---------------- tail ----------------
        proj_q.extend([(3, q2, u) for q2 in range(2) for u in range(16)])
        while proj_q:
            proj_step(*proj_q.pop(0))

    nc.compile()
    return nc


def _in_maps(x, w_weight, w_bias, proj_weight, proj_bias):
    x = np.ascontiguousarray(x, np.float32)
    w_weight = np.ascontiguousarray(w_weight, np.float32)
    w_bias = np.ascontiguousarray(w_bias, np.float32)
    proj_weight = np.ascontiguousarray(proj_weight, np.float32)
    proj_bias = np.ascontiguousarray(proj_bias, np.float32)

    wpT = np.ascontiguousarray(
        proj_weight.T.reshape(16, 64, 1024).transpose(1, 0, 2).astype(ml_dtypes.bfloat16))
    pbr = np.ascontiguousarray(np.tile(proj_bias[None], (128, 1)))
    vones = np.ones((128, 1), dtype=ml_dtypes.bfloat16)

    maps = []
    for c in range(NCORES):
        b = c // 2
        h0 = (c % 2) * HPC
        # x[b] -> [part(c%128), ct, t] in eighth-of-t chunks
        xT = x[b].T.reshape(8, 128, 2048).transpose(1, 0, 2)  # [p, ct, t]
        xqc = np.ascontiguousarray(
            xT.reshape(128, 8, 8, 256).transpose(2, 0, 1, 3)
            .reshape(8, 128, 2048).astype(ml_dtypes.bfloat16))
        # K tiles (mt 0-3), Q tiles (mt 4-7): [mt][p, ct*128 + m]
        wk = w_weight[h0 * 64: h0 * 64 + 512]
        wq = w_weight[1024 + h0 * 64: 1024 + h0 * 64 + 512]
        wkqc = np.zeros((8, 128, 1024), np.float32)
        for mt in range(4):
            for src, off in ((wk, 0), (wq, 4)):
                rows = src[mt * 128:(mt + 1) * 128]  # [128m, 1024c]
                wkqc[mt + off] = rows.T.reshape(8, 128, 128).transpose(
                    1, 0, 2).reshape(128, 1024)
        wkqc = np.ascontiguousarray(wkqc.astype(ml_dtypes.bfloat16))
        wv = w_weight[2048 + h0 * 64: 2048 + h0 * 64 + 512]  # [512m, 1024c]
        wvc = np.ascontiguousarray(
            wv.T.reshape(8, 128, 512).transpose(1, 0, 2)
            .reshape(128, 4096).astype(ml_dtypes.bfloat16))
        bk = w_bias[h0 * 64: h0 * 64 + 512]
        bq = w_bias[1024 + h0 * 64: 1024 + h0 * 64 + 512]
        bvc = w_bias[2048 + h0 * 64: 2048 + h0 * 64 + 512]
        bkqc = np.ascontiguousarray(
            np.concatenate([bk.reshape(4, 128).T, bq.reshape(4, 128).T], axis=1))
        bvr = np.ascontiguousarray(np.tile(bvc[None], (128, 1)))
        maps.append({
            "xq": xqc, "wkq": wkqc, "wvd": wvc, "bkq": bkqc, "bv": bvr,
            "wp": wpT, "pb": pbr, "vones": vones,
            "onesr": np.ones((1, 64), np.float32),
        })
    return maps


def _install_ntff_hook():
    """Register the axon NTFF profiling hook (missing antenv.axon_hooks shim)."""
    import contextlib
    import ctypes
    import types

    if "antenv.axon_hooks" in sys.modules:
        return
    import antenv
    so_path = "/opt/axon/libaxon_pjrt.so"
    try:
        lib = ctypes.CDLL(so_path)
    except OSError:
        return
    if not hasattr(lib, "axon_start_nrt_profile"):
        return
    lib.axon_start_nrt_profile.argtypes = [ctypes.POINTER(ctypes.c_int64),
                                           ctypes.c_size_t]
    lib.axon_start_nrt_profile.restype = ctypes.c_int64
    lib.axon_stop_nrt_profile.argtypes = [ctypes.c_char_p]
    lib.axon_stop_nrt_profile.restype = ctypes.c_int64

    @contextlib.contextmanager
    def _hook(output_dir, device_ids):
        import jax
        jax.devices()
        if device_ids:
            ids = (ctypes.c_int64 * len(device_ids))(*device_ids)
            rc = lib.axon_start_nrt_profile(ids, len(device_ids))
        else:
            rc = lib.axon_start_nrt_profile(None, 0)
        if rc != 0:
            raise RuntimeError(f"axon_start_nrt_profile rc={rc}")
        try:
            yield
        finally:
            n = lib.axon_stop_nrt_profile(str(output_dir).encode())
            print(f"profile: {n} file(s) written to {output_dir}", file=sys.stderr)

    mod = types.ModuleType("antenv.axon_hooks")
    mod.get_axon_ntff_profile_hook = lambda: _hook
    mod.set_axon_ntff_profile_hook = lambda h: None
    sys.modules["antenv.axon_hooks"] = mod
    antenv.axon_hooks = mod


def _run(x, w_weight, w_bias, proj_weight, proj_bias, trace=False):
    from concourse.bass_utils import run_bass_kernel_spmd

    if trace:
        _install_ntff_hook()

    if "nc" not in _CACHE:
        _CACHE["nc"] = _build()
    nc = _CACHE["nc"]
    maps = _in_maps(x, w_weight, w_bias, proj_weight, proj_bias)
    res = run_bass_kernel_spmd(nc, maps, core_ids=list(range(NCORES)), trace=trace)
    out = np.zeros((B, T, C), np.float32)
    for c in range(NCORES):
        yc = res.results[c]["y"]  # [128, 8, 1024]
        b = c // 2
        h0 = (c % 2) * HPC
        for j in range(HPC):
            out[b, (h0 + j) * 128:(h0 + j + 1) * 128, :] = yc[:, j, :]
    return out, res.exec_time_ns


def kernel(x, w_weight, w_bias, proj_weight, proj_bias):
    out, _ = _run(x, w_weight, w_bias, proj_weight, proj_bias, trace=False)
    return out


def kernel_with_time(x, w_weight, w_bias, proj_weight, proj_bias):
    return _run(x, w_weight, w_bias, proj_weight, proj_bias, trace=True)


# revision 39
# speedup vs baseline: 1.1687x; 1.1687x over previous
"""Fused attention kernel for Trainium2, 8 NeuronCores.

Problem: B=4, T=2048, C=1024, nh=16, hs=64, fused QKV (chunk order k,q,v),
softmax attention, then (faithful reference bug) reshape (B,nh,T,hs)->(B,T,C)
directly before the output projection.

Key structural fact: with the buggy reshape, head h's attention output
occupies exactly rows [h*128, (h+1)*128) of the reshaped (T, C) matrix
(row tau = h*128 + t//16, col = (t%16)*64 + d). So everything after the
QKV projection is fully independent per (batch, head) pair; the output
projection needs no cross-head reduction.

Sharding: 8 cores = 4 batches x 2 head-groups (8 heads each). Each core
computes its batch's QKV slice and its 8 heads end-to-end. No collectives.

v2: single fused pipeline. The whole kernel is one stream of 256
attention iterations (hp, ic, j) with the QKV projection folded in as
prefix (K0,Q0,V) + per-block filler bursts, so the tensor engine never
drains and the scalar engine (exp) starts ~25us in. Scores operands in
bf16 (fp32 rhs streams ~1.5x slower). A fraction of the exp tiles run
on the DVE via a Schraudolph int16 bit-trick (exp error is a ~2% sawtooth
whose constant factor cancels in softmax normalization), keeping the
scalar engine below the tensor engine's per-iteration cost.
"""

import math
import sys

import numpy as np

sys.path.insert(0, "/opt/trn_rl_repo")

import ml_dtypes  # noqa: E402

B, T, C = 4, 2048, 1024
NH, HS = 16, 64
NCORES = 8
HPC = 8  # heads per core

_CACHE = {}

# Schraudolph fast-exp constants (bf16 bit pattern via int16):
# exp(0.125*x) ~= bitcast_bf16(int16(x * (0.125*128/ln2) + B)). B is shifted
# below 127*128=16256 to zero the mean of the sawtooth approximation error:
# a nonzero mean is a systematic softmax-weight bias for the key-blocks that
# take this path (the constant factor only cancels when ALL blocks share it).
EXP_A = 0.125 * 128.0 / math.log(2.0)
EXP_B = 16249.8


def _build():
    from contextlib import ExitStack

    import concourse.bass as bass  # noqa: F401
    import concourse.mybir as mybir
    from concourse import bacc, tile

    F32 = mybir.dt.float32
    BF16 = mybir.dt.bfloat16
    I16 = mybir.dt.int16
    ADD = mybir.AluOpType.add
    MULT = mybir.AluOpType.mult
    EXP = mybir.ActivationFunctionType.Exp
    IDENT = mybir.ActivationFunctionType.Identity

    nc = bacc.Bacc()
    # DRAM inputs (host-prepacked for contiguous, full-rate DMA rows)
    xq = nc.dram_tensor("xq", [8, 128, 2048], BF16, kind="ExternalInput")
    wkq = nc.dram_tensor("wkq", [8, 128, 1024], BF16, kind="ExternalInput")
    wvd = nc.dram_tensor("wvd", [128, 4096], BF16, kind="ExternalInput")
    bkq = nc.dram_tensor("bkq", [128, 8], F32, kind="ExternalInput")
    bv = nc.dram_tensor("bv", [128, 512], F32, kind="ExternalInput")
    wp = nc.dram_tensor("wp", [64, 16, 1024], BF16, kind="ExternalInput")
    pb = nc.dram_tensor("pb", [128, 1024], F32, kind="ExternalInput")
    vones = nc.dram_tensor("vones", [128, 1], BF16, kind="ExternalInput")
    onesr = nc.dram_tensor("onesr", [1, 64], F32, kind="ExternalInput")
    y = nc.dram_tensor("y", [128, 8, 1024], F32, kind="ExternalOutput")

    with tile.TileContext(nc) as tc, ExitStack() as ctx:
        persist = ctx.enter_context(tc.tile_pool(name="persist", bufs=1))
        utp = ctx.enter_context(tc.tile_pool(name="utp", bufs=10))
        usp = ctx.enter_context(tc.tile_pool(name="usp", bufs=2))
        nrm = ctx.enter_context(tc.tile_pool(name="nrm", bufs=2))
        yps = ctx.enter_context(tc.tile_pool(name="ysb", bufs=2))
        spx = ctx.enter_context(tc.tile_pool(name="spool", bufs=2, space="PSUM"))
        ypx = ctx.enter_context(tc.tile_pool(name="ypool", bufs=2, space="PSUM"))
        opx = ctx.enter_context(tc.tile_pool(name="opool", bufs=1, space="PSUM"))
        rpx = ctx.enter_context(tc.tile_pool(name="rpool", bufs=1, space="PSUM"))
        dpool = ctx.enter_context(tc.tile_pool(name="dpool", bufs=2, space="DRAM"))

        # ---- persistent tiles + priority-ordered input DMAs ----
        wkq_sb = [persist.tile([128, 1024], BF16, tag=f"wkq{mt}", name=f"wkq{mt}")
                  for mt in range(8)]
        xts = persist.tile([128, 8, 2048], BF16, tag="xts")
        wv_sb = persist.tile([128, 8, 512], BF16, tag="wv")
        bkq_sb = persist.tile([128, 8], F32, tag="bkq")
        bv_sb = persist.tile([128, 512], F32, tag="bv")
        vones_sb = persist.tile([128, 1], BF16, tag="vones")
        wp_sb = persist.tile([128, 16, 1024], BF16, tag="wp")
        pb_sb = persist.tile([128, 1024], F32, tag="pb")
        vbuf = persist.tile([128, 16, HPC, 64], BF16, tag="vbuf")
        qk = [persist.tile([128, 2048], BF16, tag=f"qk{mt}", name=f"qk{mt}")
              for mt in range(8)]
        ots = [persist.tile([128, 2048], BF16, tag=f"ot{hp}", name=f"ot{hp}")
               for hp in range(4)]

        # first-needed first: K0 weights, x half-quarters, Q0/K1/Q1 weights,
        # V weights, remaining KQ weights, then attention-phase constants.
        nc.sync.dma_start(wkq_sb[0], wkq[0])
        nc.sync.dma_start(bkq_sb, bkq[:])
        nc.sync.dma_start(
            xts[:, :, 0:256], xq[0].rearrange("p (a b) -> p a b", b=256))
        nc.sync.dma_start(
            xts[:, :, 256:512], xq[1].rearrange("p (a b) -> p a b", b=256))
        nc.sync.dma_start(wkq_sb[4], wkq[4])
        for q in range(2, 8):
            nc.sync.dma_start(
                xts[:, :, q * 256:(q + 1) * 256],
                xq[q].rearrange("p (a b) -> p a b", b=256))
        nc.sync.dma_start(wkq_sb[1], wkq[1])
        nc.sync.dma_start(wkq_sb[5], wkq[5])
        nc.sync.dma_start(wv_sb, wvd.rearrange("p (a b) -> p a b", b=512))
        nc.sync.dma_start(bv_sb, bv[:])
        for mt in (2, 6, 3, 7):
            nc.sync.dma_start(wkq_sb[mt], wkq[mt])
        nc.sync.dma_start(vones_sb, vones[:])
        onesr_sb = persist.tile([1, 64], F32, tag="onesr")
        nc.sync.dma_start(onesr_sb, onesr[:])
        nc.sync.dma_start(wp_sb[0:64], wp[:])
        nc.sync.dma_start(wp_sb[64:128], wp[:])
        nc.sync.dma_start(pb_sb, pb[:])

        # ---------------- emission helpers ----------------
        ITEMS = [(hp, ic, j) for hp in range(4) for ic in range(4)
                 for j in range(16)]
        DVE_J = {3, 7, 11, 15}
        uts = {}
        usums = {}

        def kq_group(mt, ic2, chunk=512):
            ps = spx.tile([128, 1024], F32, tag="sp", name=f"kq{mt}_{ic2}")
            for half in range(1024 // chunk):
                t0 = ic2 * 1024 + half * chunk
                for ct in range(8):
                    nc.tensor.matmul(
                        ps[:, half * chunk:(half + 1) * chunk],
                        wkq_sb[mt][:, ct * 128:(ct + 1) * 128],
                        xts[:, ct, t0:t0 + chunk],
                        start=(ct == 0), stop=(ct == 7))
            # Identity shares the exp_and_others act table: no table thrash
            nc.scalar.activation(
                qk[mt][:, ic2 * 1024:(ic2 + 1) * 1024], ps, IDENT,
                bias=bkq_sb[:, mt:mt + 1])

        def v_group(tt):
            ps = ypx.tile([128, 512], F32, tag="yp", name=f"v{tt}")
            for ct in range(8):
                nc.tensor.matmul(
                    ps, xts[:, ct, tt * 128:(tt + 1) * 128], wv_sb[:, ct, :],
                    start=(ct == 0), stop=(ct == 7))
            nc.vector.tensor_tensor(
                vbuf[:, tt, :, :],
                ps.rearrange("p (h d) -> p h d", d=64),
                bv_sb.rearrange("p (h d) -> p h d", d=64), ADD)

        def emit_score(idx):
            hp, ic, j = ITEMS[idx]
            kt = qk[hp]
            qt = qk[4 + hp]
            jsl = slice(j * 128, (j + 1) * 128)
            isl = slice(ic * 512, (ic + 1) * 512)
            sp = spx.tile([128, 1024], F32, tag="sp", name="sp")
            nc.tensor.matmul(sp[:, 0:512], kt[0:64, jsl], qt[0:64, isl],
                             start=True, stop=True)
            nc.tensor.matmul(sp[:, 512:1024], kt[64:128, jsl],
                             qt[64:128, isl], start=True, stop=True)
            ut = utp.tile([128, 1024], BF16, tag="ut", name="ut")
            if j in DVE_J:
                nc.vector.tensor_scalar(
                    ut.bitcast(I16), sp, EXP_A, EXP_B, MULT, ADD)
            else:
                nc.scalar.activation(ut, sp, EXP, scale=0.125)
            uts[idx] = ut
            # pairwise-tree partial sums of exp tiles on the DVE: one
            # denominator matmul pair per 4 key-tiles instead of per 1.
            if j % 4 == 1:
                s1 = usp.tile([128, 1024], BF16, tag="us1", name="us1")
                nc.vector.tensor_tensor(s1, uts[idx - 1], ut, ADD)
                usums[(idx - 1) // 4] = s1
            elif j % 4 == 3:
                s2 = usp.tile([128, 1024], BF16, tag="us1", name="us2")
                nc.vector.tensor_tensor(s2, uts[idx - 1], ut, ADD)
                s12 = usp.tile([128, 1024], BF16, tag="us12", name="us12")
                nc.vector.tensor_tensor(s12, usums[(idx - 3) // 4], s2, ADD)
                usums[(idx - 3) // 4] = s12

        av_state = {}

        def emit_av(idx):
            hp, ic, j = ITEMS[idx]
            hA, hB = 2 * hp, 2 * hp + 1
            if j == 0:
                av_state["op"] = opx.tile([128, 512], F32, tag="op", name="op")
                av_state["rs"] = rpx.tile([33, 512], F32, tag="rs", name="rs")
            optile = av_state["op"]
            ut = uts.pop(idx)
            nc.tensor.matmul(optile[0:64, :], vbuf[:, j, hA, :], ut[:, 0:512],
                             start=(j == 0), stop=(j == 15))
            nc.tensor.matmul(optile[64:128, :], vbuf[:, j, hB, :],
                             ut[:, 512:1024],
                             start=(j == 0), stop=(j == 15),
                             tile_position=(0, 64))

        def emit_ones_quad(idx):
            # idx = last item of a quad; sums exp over key-tiles 4qd..4qd+3
            qd = (idx % 16) // 4
            rsps = av_state["rs"]
            us = usums.pop(idx // 4)
            nc.tensor.matmul(rsps[0:1, :], vones_sb, us[:, 0:512],
                             start=(qd == 0), stop=(qd == 3))
            nc.tensor.matmul(rsps[32:33, :], vones_sb, us[:, 512:1024],
                             start=(qd == 0), stop=(qd == 3),
                             tile_position=(0, 32))

        norm_state = {}

        def norm_stage1a(hp, ic):
            # osb copy gates the op-psum slot for the next block's AV j=0:
            # emit it first, straight after the block's last AV matmul.
            osb = nrm.tile([128, 512], F32, tag="osb", name="osb")
            nc.vector.tensor_copy(osb, av_state["op"])
            return osb

        def norm_stage1(hp, ic, osb):
            rsps = av_state["rs"]
            rsb = nrm.tile([33, 512], F32, tag="rsb", name="rsb")
            nc.vector.tensor_copy(rsb, rsps)
            scr1 = dpool.tile([1024], F32, tag="scr1", name="scr1")
            nc.sync.dma_start(
                scr1.rearrange("(r f) -> r f", r=2), rsb[0:33:32, :])
            rst = nrm.tile([128, 8], F32, tag="rst", name="rst")
            nc.sync.dma_start(rst, scr1.rearrange("(p f) -> p f", f=8))
            norm_state[(hp, ic)] = (osb, rst)

        def norm_tail_fast(hp, ic):
            # last block: skip the DRAM broadcast round-trip; reciprocal on
            # the row copies, partition-broadcast via a PE outer product into
            # now-free PSUM, multiply from there.
            optile, rsps = av_state["op"], av_state["rs"]
            rsb = nrm.tile([33, 512], F32, tag="rsb", name="rsb")
            nc.vector.tensor_copy(rsb, rsps)
            osb = nrm.tile([128, 512], F32, tag="osb", name="osb")
            nc.vector.tensor_copy(osb, optile)
            rsbB = persist.tile([1, 512], F32, tag="rsbB", name="rsbB")
            nc.sync.dma_start(rsbB, rsb[32:33, :])
            nc.vector.reciprocal(rsb[0:1, :], rsb[0:1, :])
            nc.vector.reciprocal(rsbB, rsbB)
            bc = spx.tile([128, 1024], F32, tag="sp", name="bc")
            nc.tensor.matmul(bc[0:64, 0:512], onesr_sb, rsb[0:1, :],
                             start=True, stop=True)
            nc.tensor.matmul(bc[64:128, 0:512], onesr_sb, rsbB,
                             start=True, stop=True, tile_position=(0, 64))
            nc.vector.tensor_tensor(
                ots[hp][:, ic * 512:(ic + 1) * 512], osb, bc[:, 0:512], MULT)

        def norm_stage2(hp, ic):
            osb, rst = norm_state[(hp, ic)]
            nc.vector.reciprocal(rst, rst)
            scr2 = dpool.tile([1024], F32, tag="scr2", name="scr2")
            nc.sync.dma_start(scr2.rearrange("(p f) -> p f", f=8), rst)
            bcsb = nrm.tile([128, 512], F32, tag="bcsb", name="bcsb")
            nc.sync.dma_start(
                bcsb[0:64, :], scr2[None, 0:512].to_broadcast((64, 512)))
            nc.sync.dma_start(
                bcsb[64:128, :], scr2[None, 512:1024].to_broadcast((64, 512)))
            norm_state[(hp, ic)] = (osb, bcsb)

        def norm_stage3(hp, ic):
            osb, bcsb = norm_state.pop((hp, ic))
            nc.vector.tensor_tensor(
                ots[hp][:, ic * 512:(ic + 1) * 512], osb, bcsb, MULT)

        proj_state = {}

        def proj_step(hp, q2, u):
            if "ypA" not in proj_state:
                proj_state["ypA"] = ypx.tile([128, 512], F32, tag="yp",
                                             name="ypA")
                proj_state["ypB"] = ypx.tile([128, 512], F32, tag="yp",
                                             name="ypB")
            ypA, ypB = proj_state["ypA"], proj_state["ypB"]
            otr = ots[hp].rearrange("d (t u) -> d u t", u=16)
            csl = slice(q2 * 512, (q2 + 1) * 512)
            nc.tensor.matmul(ypA, otr[0:64, u, :], wp_sb[0:64, u, csl],
                             start=(u == 0), stop=(u == 15))
            nc.tensor.matmul(ypB, otr[64:128, u, :], wp_sb[64:128, u, csl],
                             start=(u == 0), stop=(u == 15))
            if u == 15:
                for h, yp in ((2 * hp, ypA), (2 * hp + 1, ypB)):
                    ysb = yps.tile([128, 512], F32, tag="ysb", name="ysb")
                    nc.vector.tensor_tensor(ysb, yp, pb_sb[:, csl], ADD)
                    nc.sync.dma_start(y[:, h, csl], ysb)
                proj_state.clear()

        # ---------------- prefix: K0,Q0,K1,Q1, V, first scores -----------
        LOOK = 8
        nscore = 0

        def next_score():
            nonlocal nscore
            emit_score(nscore)
            nscore += 1

        kq_group(0, 0, chunk=256)
        kq_group(4, 0)
        kq_group(0, 1)
        kq_group(4, 1)
        kq_group(1, 0)
        next_score()
        kq_group(5, 0)
        next_score()
        kq_group(1, 1)
        next_score()
        kq_group(5, 1)
        next_score()
        for tt in range(16):
            v_group(tt)
            if tt in (5, 8, 11, 14) and nscore < LOOK:
                next_score()

        # ------------- main loop: 16 blocks x 8 groups of 2 iters --------
        FILLER = [(2, 0), (2, 1), (6, 0), (6, 1), (3, 0), (3, 1),
                  (7, 0), (7, 1)]
        BLOCKS = [(hp, ic) for hp in range(4) for ic in range(4)]
        proj_q = []
        prev_block = None

        for bi, (hp, ic) in enumerate(BLOCKS):
            if bi < 8:
                kq_group(*FILLER[bi])
            for g in range(8):
                w = 16 * bi + 2 * g
                if proj_q:
                    proj_step(*proj_q.pop(0))
                    if len(proj_q) > 8:
                        proj_step(*proj_q.pop(0))
                if nscore < 256:
                    next_score()
                if nscore < 256:
                    next_score()
                emit_av(w)
                emit_av(w + 1)
                if g == 7 and bi < 15:
                    osb = norm_stage1a(hp, ic)
                if g % 2 == 1:
                    emit_ones_quad(w + 1)
                if prev_block is not None:
                    if g == 2:
                        norm_stage2(*prev_block)
                    elif g == 4:
                        norm_stage3(*prev_block)
                    elif g == 5 and prev_block[1] == 3:
                        php = prev_block[0]
                        proj_q.extend([(php, q2, u) for q2 in range(2)
                                       for u in range(16)])
            if bi < 15:
                norm_stage1(hp, ic, osb)
            else:
                norm_tail_fast(hp, ic)
            prev_block = (hp, ic)

        # ---------------- tail ----------------
        proj_q.extend([(3, q2, u) for q2 in range(2) for u in range(16)])
        while proj_q:
            proj_step(*proj_q.pop(0))

    nc.compile()
    return nc


def _in_maps(x, w_weight, w_bias, proj_weight, proj_bias):
    x = np.ascontiguousarray(x, np.float32)
    w_weight = np.ascontiguousarray(w_weight, np.float32)
    w_bias = np.ascontiguousarray(w_bias, np.float32)
    proj_weight = np.ascontiguousarray(proj_weight, np.float32)
    proj_bias = np.ascontiguousarray(proj_bias, np.float32)

    wpT = np.ascontiguousarray(
        proj_weight.T.reshape(16, 64, 1024).transpose(1, 0, 2).astype(ml_dtypes.bfloat16))
    pbr = np.ascontiguousarray(np.tile(proj_bias[None], (128, 1)))
    vones = np.ones((128, 1), dtype=ml_dtypes.bfloat16)

    maps = []
    for c in range(NCORES):
        b = c // 2
        h0 = (c % 2) * HPC
        # x[b] -> [part(c%128), ct, t] in eighth-of-t chunks
        xT = x[b].T.reshape(8, 128, 2048).transpose(1, 0, 2)  # [p, ct, t]
        xqc = np.ascontiguousarray(
            xT.reshape(128, 8, 8, 256).transpose(2, 0, 1, 3)
            .reshape(8, 128, 2048).astype(ml_dtypes.bfloat16))
        # K tiles (mt 0-3), Q tiles (mt 4-7): [mt][p, ct*128 + m]
        wk = w_weight[h0 * 64: h0 * 64 + 512]
        wq = w_weight[1024 + h0 * 64: 1024 + h0 * 64 + 512]
        wkqc = np.zeros((8, 128, 1024), np.float32)
        for mt in range(4):
            for src, off in ((wk, 0), (wq, 4)):
                rows = src[mt * 128:(mt + 1) * 128]  # [128m, 1024c]
                wkqc[mt + off] = rows.T.reshape(8, 128, 128).transpose(
                    1, 0, 2).reshape(128, 1024)
        wkqc = np.ascontiguousarray(wkqc.astype(ml_dtypes.bfloat16))
        wv = w_weight[2048 + h0 * 64: 2048 + h0 * 64 + 512]  # [512m, 1024c]
        wvc = np.ascontiguousarray(
            wv.T.reshape(8, 128, 512).transpose(1, 0, 2)
            .reshape(128, 4096).astype(ml_dtypes.bfloat16))
        bk = w_bias[h0 * 64: h0 * 64 + 512]
        bq = w_bias[1024 + h0 * 64: 1024 + h0 * 64 + 512]
        bvc = w_bias[2048 + h0 * 64: 2048 + h0 * 64 + 512]
        bkqc = np.ascontiguousarray(
            np.concatenate([bk.reshape(4, 128).T, bq.reshape(4, 128).T], axis=1))
        bvr = np.ascontiguousarray(np.tile(bvc[None], (128, 1)))
        maps.append({
            "xq": xqc, "wkq": wkqc, "wvd": wvc, "bkq": bkqc, "bv": bvr,
            "wp": wpT, "pb": pbr, "vones": vones,
            "onesr": np.ones((1, 64), np.float32),
        })
    return maps


def _install_ntff_hook():
    """Register the axon NTFF profiling hook (missing antenv.axon_hooks shim)."""
    import contextlib
    import ctypes
    import types

    if "antenv.axon_hooks" in sys.modules:
        return
    import antenv
    so_path = "/opt/axon/libaxon_pjrt.so"
    try:
        lib = ctypes.CDLL(so_path)
    except OSError:
        return
    if not hasattr(lib, "axon_start_nrt_profile"):
        return
    lib.axon_start_nrt_profile.argtypes = [ctypes.POINTER(ctypes.c_int64),
                                           ctypes.c_size_t]
    lib.axon_start_nrt_profile.restype = ctypes.c_int64
    lib.axon_stop_nrt_profile.argtypes = [ctypes.c_char_p]
    lib.axon_stop_nrt_profile.restype = ctypes.c_int64

    @contextlib.contextmanager
    def _hook(output_dir, device_ids):
        import jax
        jax.devices()
        if device_ids:
            ids = (ctypes.c_int64 * len(device_ids))(*device_ids)
            rc = lib.axon_start_nrt_profile(ids, len(device_ids))
        else:
            rc = lib.axon_start_nrt_profile(None, 0)
        if rc != 0:
            raise RuntimeError(f"axon_start_nrt_profile rc={rc}")
        try:
            yield
        finally:
            n = lib.axon_stop_nrt_profile(str(output_dir).encode())
            print(f"profile: {n} file(s) written to {output_dir}", file=sys.stderr)

    mod = types.ModuleType("antenv.axon_hooks")
    mod.get_axon_ntff_profile_hook = lambda: _hook
    mod.set_axon_ntff_profile_hook = lambda h: None
    sys.modules["antenv.axon_hooks"] = mod
    antenv.axon_hooks = mod


def _run(x, w_weight, w_bias, proj_weight, proj_bias, trace=False):
    from concourse.bass_utils import run_bass_kernel_spmd

    if trace:
        _install_ntff_hook()

    if "nc" not in _CACHE:
        _CACHE["nc"] = _build()
    nc = _CACHE["nc"]
    maps = _in_maps(x, w_weight, w_bias, proj_weight, proj_bias)
    res = run_bass_kernel_spmd(nc, maps, core_ids=list(range(NCORES)), trace=trace)
    out = np.zeros((B, T, C), np.float32)
    for c in range(NCORES):
        yc = res.results[c]["y"]  # [128, 8, 1024]
        b = c // 2
        h0 = (c % 2) * HPC
        for j in range(HPC):
            out[b, (h0 + j) * 128:(h0 + j + 1) * 128, :] = yc[:, j, :]
    return out, res.exec_time_ns


def kernel(x, w_weight, w_bias, proj_weight, proj_bias):
    out, _ = _run(x, w_weight, w_bias, proj_weight, proj_bias, trace=False)
    return out


def kernel_with_time(x, w_weight, w_bias, proj_weight, proj_bias):
    return _run(x, w_weight, w_bias, proj_weight, proj_bias, trace=True)


# revision 40
# speedup vs baseline: 1.1711x; 1.0020x over previous
"""Fused attention kernel for Trainium2, 8 NeuronCores.

Problem: B=4, T=2048, C=1024, nh=16, hs=64, fused QKV (chunk order k,q,v),
softmax attention, then (faithful reference bug) reshape (B,nh,T,hs)->(B,T,C)
directly before the output projection.

Key structural fact: with the buggy reshape, head h's attention output
occupies exactly rows [h*128, (h+1)*128) of the reshaped (T, C) matrix
(row tau = h*128 + t//16, col = (t%16)*64 + d). So everything after the
QKV projection is fully independent per (batch, head) pair; the output
projection needs no cross-head reduction.

Sharding: 8 cores = 4 batches x 2 head-groups (8 heads each). Each core
computes its batch's QKV slice and its 8 heads end-to-end. No collectives.

v2: single fused pipeline. The whole kernel is one stream of 256
attention iterations (hp, ic, j) with the QKV projection folded in as
prefix (K0,Q0,V) + per-block filler bursts, so the tensor engine never
drains and the scalar engine (exp) starts ~25us in. Scores operands in
bf16 (fp32 rhs streams ~1.5x slower). A fraction of the exp tiles run
on the DVE via a Schraudolph int16 bit-trick (exp error is a ~2% sawtooth
whose constant factor cancels in softmax normalization), keeping the
scalar engine below the tensor engine's per-iteration cost.
"""

import math
import sys

import numpy as np

sys.path.insert(0, "/opt/trn_rl_repo")

import ml_dtypes  # noqa: E402

B, T, C = 4, 2048, 1024
NH, HS = 16, 64
NCORES = 8
HPC = 8  # heads per core

_CACHE = {}

# Schraudolph fast-exp constants (bf16 bit pattern via int16):
# exp(0.125*x) ~= bitcast_bf16(int16(x * (0.125*128/ln2) + B)). B is shifted
# below 127*128=16256 to zero the mean of the sawtooth approximation error:
# a nonzero mean is a systematic softmax-weight bias for the key-blocks that
# take this path (the constant factor only cancels when ALL blocks share it).
EXP_A = 0.125 * 128.0 / math.log(2.0)
EXP_B = 16249.8


def _build():
    from contextlib import ExitStack

    import concourse.bass as bass  # noqa: F401
    import concourse.mybir as mybir
    from concourse import bacc, tile

    F32 = mybir.dt.float32
    BF16 = mybir.dt.bfloat16
    I16 = mybir.dt.int16
    ADD = mybir.AluOpType.add
    MULT = mybir.AluOpType.mult
    EXP = mybir.ActivationFunctionType.Exp
    IDENT = mybir.ActivationFunctionType.Identity

    nc = bacc.Bacc()
    # DRAM inputs (host-prepacked for contiguous, full-rate DMA rows)
    xq = nc.dram_tensor("xq", [8, 128, 2048], BF16, kind="ExternalInput")
    wkq = nc.dram_tensor("wkq", [8, 128, 1024], BF16, kind="ExternalInput")
    wvd = nc.dram_tensor("wvd", [128, 4096], BF16, kind="ExternalInput")
    bkq = nc.dram_tensor("bkq", [128, 8], F32, kind="ExternalInput")
    bv = nc.dram_tensor("bv", [128, 512], F32, kind="ExternalInput")
    wp = nc.dram_tensor("wp", [64, 16, 1024], BF16, kind="ExternalInput")
    pb = nc.dram_tensor("pb", [128, 1024], F32, kind="ExternalInput")
    vones = nc.dram_tensor("vones", [128, 1], BF16, kind="ExternalInput")
    onesr = nc.dram_tensor("onesr", [1, 64], F32, kind="ExternalInput")
    y = nc.dram_tensor("y", [128, 8, 1024], F32, kind="ExternalOutput")

    with tile.TileContext(nc) as tc, ExitStack() as ctx:
        persist = ctx.enter_context(tc.tile_pool(name="persist", bufs=1))
        utp = ctx.enter_context(tc.tile_pool(name="utp", bufs=10))
        usp = ctx.enter_context(tc.tile_pool(name="usp", bufs=2))
        nrm = ctx.enter_context(tc.tile_pool(name="nrm", bufs=2))
        yps = ctx.enter_context(tc.tile_pool(name="ysb", bufs=2))
        spx = ctx.enter_context(tc.tile_pool(name="spool", bufs=2, space="PSUM"))
        ypx = ctx.enter_context(tc.tile_pool(name="ypool", bufs=2, space="PSUM"))
        opx = ctx.enter_context(tc.tile_pool(name="opool", bufs=1, space="PSUM"))
        rpx = ctx.enter_context(tc.tile_pool(name="rpool", bufs=1, space="PSUM"))
        dpool = ctx.enter_context(tc.tile_pool(name="dpool", bufs=2, space="DRAM"))

        # ---- persistent tiles + priority-ordered input DMAs ----
        wkq_sb = [persist.tile([128, 1024], BF16, tag=f"wkq{mt}", name=f"wkq{mt}")
                  for mt in range(8)]
        xts = persist.tile([128, 8, 2048], BF16, tag="xts")
        wv_sb = persist.tile([128, 8, 512], BF16, tag="wv")
        bkq_sb = persist.tile([128, 8], F32, tag="bkq")
        bv_sb = persist.tile([128, 512], F32, tag="bv")
        vones_sb = persist.tile([128, 1], BF16, tag="vones")
        wp_sb = persist.tile([128, 16, 1024], BF16, tag="wp")
        pb_sb = persist.tile([128, 1024], F32, tag="pb")
        vbuf = persist.tile([128, 16, HPC, 64], BF16, tag="vbuf")
        qk = [persist.tile([128, 2048], BF16, tag=f"qk{mt}", name=f"qk{mt}")
              for mt in range(8)]
        ots = [persist.tile([128, 2048], BF16, tag=f"ot{hp}", name=f"ot{hp}")
               for hp in range(4)]

        # first-needed first: K0 weights, x half-quarters, Q0/K1/Q1 weights,
        # V weights, remaining KQ weights, then attention-phase constants.
        nc.sync.dma_start(wkq_sb[0], wkq[0])
        nc.sync.dma_start(bkq_sb, bkq[:])
        nc.sync.dma_start(
            xts[:, :, 0:256], xq[0].rearrange("p (a b) -> p a b", b=256))
        nc.sync.dma_start(
            xts[:, :, 256:512], xq[1].rearrange("p (a b) -> p a b", b=256))
        nc.sync.dma_start(wkq_sb[4], wkq[4])
        for q in range(2, 8):
            nc.sync.dma_start(
                xts[:, :, q * 256:(q + 1) * 256],
                xq[q].rearrange("p (a b) -> p a b", b=256))
        nc.sync.dma_start(wkq_sb[1], wkq[1])
        nc.sync.dma_start(wkq_sb[5], wkq[5])
        nc.sync.dma_start(wv_sb, wvd.rearrange("p (a b) -> p a b", b=512))
        nc.sync.dma_start(bv_sb, bv[:])
        for mt in (2, 6, 3, 7):
            nc.sync.dma_start(wkq_sb[mt], wkq[mt])
        nc.sync.dma_start(vones_sb, vones[:])
        onesr_sb = persist.tile([1, 64], F32, tag="onesr")
        nc.sync.dma_start(onesr_sb, onesr[:])
        nc.sync.dma_start(wp_sb[0:64], wp[:])
        nc.sync.dma_start(wp_sb[64:128], wp[:])
        nc.sync.dma_start(pb_sb, pb[:])

        # ---------------- emission helpers ----------------
        ITEMS = [(hp, ic, j) for hp in range(4) for ic in range(4)
                 for j in range(16)]
        DVE_J = {3, 7, 11, 15}
        DVE_J3 = {3, 5, 7, 9, 11, 15}
        uts = {}
        usums = {}

        def kq_group(mt, ic2, chunk=512):
            ps = spx.tile([128, 1024], F32, tag="sp", name=f"kq{mt}_{ic2}")
            for half in range(1024 // chunk):
                t0 = ic2 * 1024 + half * chunk
                for ct in range(8):
                    nc.tensor.matmul(
                        ps[:, half * chunk:(half + 1) * chunk],
                        wkq_sb[mt][:, ct * 128:(ct + 1) * 128],
                        xts[:, ct, t0:t0 + chunk],
                        start=(ct == 0), stop=(ct == 7))
            # Identity shares the exp_and_others act table: no table thrash
            nc.scalar.activation(
                qk[mt][:, ic2 * 1024:(ic2 + 1) * 1024], ps, IDENT,
                bias=bkq_sb[:, mt:mt + 1])

        def v_group(tt):
            ps = ypx.tile([128, 512], F32, tag="yp", name=f"v{tt}")
            for ct in range(8):
                nc.tensor.matmul(
                    ps, xts[:, ct, tt * 128:(tt + 1) * 128], wv_sb[:, ct, :],
                    start=(ct == 0), stop=(ct == 7))
            nc.vector.tensor_tensor(
                vbuf[:, tt, :, :],
                ps.rearrange("p (h d) -> p h d", d=64),
                bv_sb.rearrange("p (h d) -> p h d", d=64), ADD)

        def emit_score(idx):
            hp, ic, j = ITEMS[idx]
            kt = qk[hp]
            qt = qk[4 + hp]
            jsl = slice(j * 128, (j + 1) * 128)
            isl = slice(ic * 512, (ic + 1) * 512)
            sp = spx.tile([128, 1024], F32, tag="sp", name="sp")
            nc.tensor.matmul(sp[:, 0:512], kt[0:64, jsl], qt[0:64, isl],
                             start=True, stop=True)
            nc.tensor.matmul(sp[:, 512:1024], kt[64:128, jsl],
                             qt[64:128, isl], start=True, stop=True)
            ut = utp.tile([128, 1024], BF16, tag="ut", name="ut")
            if j in (DVE_J3 if hp == 3 else DVE_J):
                nc.vector.tensor_scalar(
                    ut.bitcast(I16), sp, EXP_A, EXP_B, MULT, ADD)
            else:
                nc.scalar.activation(ut, sp, EXP, scale=0.125)
            uts[idx] = ut
            # pairwise-tree partial sums of exp tiles on the DVE: one
            # denominator matmul pair per 4 key-tiles instead of per 1.
            if j % 4 == 1:
                s1 = usp.tile([128, 1024], BF16, tag="us1", name="us1")
                nc.vector.tensor_tensor(s1, uts[idx - 1], ut, ADD)
                usums[(idx - 1) // 4] = s1
            elif j % 4 == 3:
                s2 = usp.tile([128, 1024], BF16, tag="us1", name="us2")
                nc.vector.tensor_tensor(s2, uts[idx - 1], ut, ADD)
                s12 = usp.tile([128, 1024], BF16, tag="us12", name="us12")
                nc.vector.tensor_tensor(s12, usums[(idx - 3) // 4], s2, ADD)
                usums[(idx - 3) // 4] = s12

        av_state = {}

        def emit_av(idx):
            hp, ic, j = ITEMS[idx]
            hA, hB = 2 * hp, 2 * hp + 1
            if j == 0:
                av_state["op"] = opx.tile([128, 512], F32, tag="op", name="op")
                av_state["rs"] = rpx.tile([33, 512], F32, tag="rs", name="rs")
            optile = av_state["op"]
            ut = uts.pop(idx)
            nc.tensor.matmul(optile[0:64, :], vbuf[:, j, hA, :], ut[:, 0:512],
                             start=(j == 0), stop=(j == 15))
            nc.tensor.matmul(optile[64:128, :], vbuf[:, j, hB, :],
                             ut[:, 512:1024],
                             start=(j == 0), stop=(j == 15),
                             tile_position=(0, 64))

        def emit_ones_quad(idx):
            # idx = last item of a quad; sums exp over key-tiles 4qd..4qd+3
            qd = (idx % 16) // 4
            rsps = av_state["rs"]
            us = usums.pop(idx // 4)
            nc.tensor.matmul(rsps[0:1, :], vones_sb, us[:, 0:512],
                             start=(qd == 0), stop=(qd == 3))
            nc.tensor.matmul(rsps[32:33, :], vones_sb, us[:, 512:1024],
                             start=(qd == 0), stop=(qd == 3),
                             tile_position=(0, 32))

        norm_state = {}

        def norm_stage1(hp, ic):
            rsps = av_state["rs"]
            rsb = nrm.tile([33, 512], F32, tag="rsb", name="rsb")
            nc.vector.tensor_copy(rsb, rsps)
            osb = nrm.tile([128, 512], F32, tag="osb", name="osb")
            nc.vector.tensor_copy(osb, av_state["op"])
            scr1 = dpool.tile([1024], F32, tag="scr1", name="scr1")
            nc.sync.dma_start(
                scr1.rearrange("(r f) -> r f", r=2), rsb[0:33:32, :])
            rst = nrm.tile([128, 8], F32, tag="rst", name="rst")
            nc.sync.dma_start(rst, scr1.rearrange("(p f) -> p f", f=8))
            norm_state[(hp, ic)] = (osb, rst)

        def norm_tail_fast(hp, ic):
            # last block: skip the DRAM broadcast round-trip; reciprocal on
            # the row copies, partition-broadcast via a PE outer product into
            # now-free PSUM, multiply from there.
            optile, rsps = av_state["op"], av_state["rs"]
            rsb = nrm.tile([33, 512], F32, tag="rsb", name="rsb")
            nc.vector.tensor_copy(rsb, rsps)
            osb = nrm.tile([128, 512], F32, tag="osb", name="osb")
            nc.vector.tensor_copy(osb, optile)
            rsbB = persist.tile([1, 512], F32, tag="rsbB", name="rsbB")
            nc.sync.dma_start(rsbB, rsb[32:33, :])
            nc.vector.reciprocal(rsb[0:1, :], rsb[0:1, :])
            nc.vector.reciprocal(rsbB, rsbB)
            bc = spx.tile([128, 1024], F32, tag="sp", name="bc")
            nc.tensor.matmul(bc[0:64, 0:512], onesr_sb, rsb[0:1, :],
                             start=True, stop=True)
            nc.tensor.matmul(bc[64:128, 0:512], onesr_sb, rsbB,
                             start=True, stop=True, tile_position=(0, 64))
            nc.vector.tensor_tensor(
                ots[hp][:, ic * 512:(ic + 1) * 512], osb, bc[:, 0:512], MULT)

        def norm_stage2(hp, ic):
            osb, rst = norm_state[(hp, ic)]
            nc.vector.reciprocal(rst, rst)
            scr2 = dpool.tile([1024], F32, tag="scr2", name="scr2")
            nc.sync.dma_start(scr2.rearrange("(p f) -> p f", f=8), rst)
            bcsb = nrm.tile([128, 512], F32, tag="bcsb", name="bcsb")
            nc.sync.dma_start(
                bcsb[0:64, :], scr2[None, 0:512].to_broadcast((64, 512)))
            nc.sync.dma_start(
                bcsb[64:128, :], scr2[None, 512:1024].to_broadcast((64, 512)))
            norm_state[(hp, ic)] = (osb, bcsb)

        def norm_stage3(hp, ic):
            osb, bcsb = norm_state.pop((hp, ic))
            nc.vector.tensor_tensor(
                ots[hp][:, ic * 512:(ic + 1) * 512], osb, bcsb, MULT)

        proj_state = {}

        def proj_step(hp, q2, u):
            if "ypA" not in proj_state:
                proj_state["ypA"] = ypx.tile([128, 512], F32, tag="yp",
                                             name="ypA")
                proj_state["ypB"] = ypx.tile([128, 512], F32, tag="yp",
                                             name="ypB")
            ypA, ypB = proj_state["ypA"], proj_state["ypB"]
            otr = ots[hp].rearrange("d (t u) -> d u t", u=16)
            csl = slice(q2 * 512, (q2 + 1) * 512)
            nc.tensor.matmul(ypA, otr[0:64, u, :], wp_sb[0:64, u, csl],
                             start=(u == 0), stop=(u == 15))
            nc.tensor.matmul(ypB, otr[64:128, u, :], wp_sb[64:128, u, csl],
                             start=(u == 0), stop=(u == 15))
            if u == 15:
                for h, yp in ((2 * hp, ypA), (2 * hp + 1, ypB)):
                    ysb = yps.tile([128, 512], F32, tag="ysb", name="ysb")
                    nc.vector.tensor_tensor(ysb, yp, pb_sb[:, csl], ADD)
                    nc.sync.dma_start(y[:, h, csl], ysb)
                proj_state.clear()

        # ---------------- prefix: K0,Q0,K1,Q1, V, first scores -----------
        LOOK = 8
        nscore = 0

        def next_score():
            nonlocal nscore
            emit_score(nscore)
            nscore += 1

        kq_group(0, 0, chunk=256)
        kq_group(4, 0)
        kq_group(0, 1)
        kq_group(4, 1)
        kq_group(1, 0)
        next_score()
        kq_group(5, 0)
        next_score()
        kq_group(1, 1)
        next_score()
        kq_group(5, 1)
        next_score()
        for tt in range(16):
            v_group(tt)
            if tt in (5, 8, 11, 14) and nscore < LOOK:
                next_score()

        # ------------- main loop: 16 blocks x 8 groups of 2 iters --------
        FILLER = [(2, 0), (2, 1), (6, 0), (6, 1), (3, 0), (3, 1),
                  (7, 0), (7, 1)]
        BLOCKS = [(hp, ic) for hp in range(4) for ic in range(4)]
        proj_q = []
        prev_block = None

        for bi, (hp, ic) in enumerate(BLOCKS):
            if bi < 8:
                kq_group(*FILLER[bi])
            for g in range(8):
                w = 16 * bi + 2 * g
                if proj_q:
                    proj_step(*proj_q.pop(0))
                    if len(proj_q) > 8:
                        proj_step(*proj_q.pop(0))
                if nscore < 256:
                    next_score()
                if nscore < 256:
                    next_score()
                emit_av(w)
                emit_av(w + 1)
                if g % 2 == 1:
                    emit_ones_quad(w + 1)
                if prev_block is not None:
                    if g == 2:
                        norm_stage2(*prev_block)
                    elif g == 4:
                        norm_stage3(*prev_block)
                    elif g == 5 and prev_block[1] == 3:
                        php = prev_block[0]
                        proj_q.extend([(php, q2, u) for q2 in range(2)
                                       for u in range(16)])
            if bi < 15:
                norm_stage1(hp, ic)
            else:
                norm_tail_fast(hp, ic)
            prev_block = (hp, ic)

        # ---------------- tail ----------------
        proj_q.extend([(3, q2, u) for q2 in range(2) for u in range(16)])
        while proj_q:
            proj_step(*proj_q.pop(0))

    nc.compile()
    return nc


def _in_maps(x, w_weight, w_bias, proj_weight, proj_bias):
    x = np.ascontiguousarray(x, np.float32)
    w_weight = np.ascontiguousarray(w_weight, np.float32)
    w_bias = np.ascontiguousarray(w_bias, np.float32)
    proj_weight = np.ascontiguousarray(proj_weight, np.float32)
    proj_bias = np.ascontiguousarray(proj_bias, np.float32)

    wpT = np.ascontiguousarray(
        proj_weight.T.reshape(16, 64, 1024).transpose(1, 0, 2).astype(ml_dtypes.bfloat16))
    pbr = np.ascontiguousarray(np.tile(proj_bias[None], (128, 1)))
    vones = np.ones((128, 1), dtype=ml_dtypes.bfloat16)

    maps = []
    for c in range(NCORES):
        b = c // 2
        h0 = (c % 2) * HPC
        # x[b] -> [part(c%128), ct, t] in eighth-of-t chunks
        xT = x[b].T.reshape(8, 128, 2048).transpose(1, 0, 2)  # [p, ct, t]
        xqc = np.ascontiguousarray(
            xT.reshape(128, 8, 8, 256).transpose(2, 0, 1, 3)
            .reshape(8, 128, 2048).astype(ml_dtypes.bfloat16))
        # K tiles (mt 0-3), Q tiles (mt 4-7): [mt][p, ct*128 + m]
        wk = w_weight[h0 * 64: h0 * 64 + 512]
        wq = w_weight[1024 + h0 * 64: 1024 + h0 * 64 + 512]
        wkqc = np.zeros((8, 128, 1024), np.float32)
        for mt in range(4):
            for src, off in ((wk, 0), (wq, 4)):
                rows = src[mt * 128:(mt + 1) * 128]  # [128m, 1024c]
                wkqc[mt + off] = rows.T.reshape(8, 128, 128).transpose(
                    1, 0, 2).reshape(128, 1024)
        wkqc = np.ascontiguousarray(wkqc.astype(ml_dtypes.bfloat16))
        wv = w_weight[2048 + h0 * 64: 2048 + h0 * 64 + 512]  # [512m, 1024c]
        wvc = np.ascontiguousarray(
            wv.T.reshape(8, 128, 512).transpose(1, 0, 2)
            .reshape(128, 4096).astype(ml_dtypes.bfloat16))
        bk = w_bias[h0 * 64: h0 * 64 + 512]
        bq = w_bias[1024 + h0 * 64: 1024 + h0 * 64 + 512]
        bvc = w_bias[2048 + h0 * 64: 2048 + h0 * 64 + 512]
        bkqc = np.ascontiguousarray(
            np.concatenate([bk.reshape(4, 128).T, bq.reshape(4, 128).T], axis=1))
        bvr = np.ascontiguousarray(np.tile(bvc[None], (128, 1)))
        maps.append({
            "xq": xqc, "wkq": wkqc, "wvd": wvc, "bkq": bkqc, "bv": bvr,
            "wp": wpT, "pb": pbr, "vones": vones,
            "onesr": np.ones((1, 64), np.float32),
        })
    return maps


def _install_ntff_hook():
    """Register the axon NTFF profiling hook (missing antenv.axon_hooks shim)."""
    import contextlib
    import ctypes
    import types

    if "antenv.axon_hooks" in sys.modules:
        return
    import antenv
    so_path = "/opt/axon/libaxon_pjrt.so"
    try:
        lib = ctypes.CDLL(so_path)
    except OSError:
        return
    if not hasattr(lib, "axon_start_nrt_profile"):
        return
    lib.axon_start_nrt_profile.argtypes = [ctypes.POINTER(ctypes.c_int64),
                                           ctypes.c_size_t]
    lib.axon_start_nrt_profile.restype = ctypes.c_int64
    lib.axon_stop_nrt_profile.argtypes = [ctypes.c_char_p]
    lib.axon_stop_nrt_profile.restype = ctypes.c_int64

    @contextlib.contextmanager
    def _hook(output_dir, device_ids):
        import jax
        jax.devices()
        if device_ids:
            ids = (ctypes.c_int64 * len(device_ids))(*device_ids)
            rc = lib.axon_start_nrt_profile(ids, len(device_ids))
        else:
            rc = lib.axon_start_nrt_profile(None, 0)
        if rc != 0:
            raise RuntimeError(f"axon_start_nrt_profile rc={rc}")
        try:
            yield
        finally:
            n = lib.axon_stop_nrt_profile(str(output_dir).encode())
            print(f"profile: {n} file(s) written to {output_dir}", file=sys.stderr)

    mod = types.ModuleType("antenv.axon_hooks")
    mod.get_axon_ntff_profile_hook = lambda: _hook
    mod.set_axon_ntff_profile_hook = lambda h: None
    sys.modules["antenv.axon_hooks"] = mod
    antenv.axon_hooks = mod


def _run(x, w_weight, w_bias, proj_weight, proj_bias, trace=False):
    from concourse.bass_utils import run_bass_kernel_spmd

    if trace:
        _install_ntff_hook()

    if "nc" not in _CACHE:
        _CACHE["nc"] = _build()
    nc = _CACHE["nc"]
    maps = _in_maps(x, w_weight, w_bias, proj_weight, proj_bias)
    res = run_bass_kernel_spmd(nc, maps, core_ids=list(range(NCORES)), trace=trace)
    out = np.zeros((B, T, C), np.float32)
    for c in range(NCORES):
        yc = res.results[c]["y"]  # [128, 8, 1024]
        b = c // 2
        h0 = (c % 2) * HPC
        for j in range(HPC):
            out[b, (h0 + j) * 128:(h0 + j + 1) * 128, :] = yc[:, j, :]
    return out, res.exec_time_ns


def kernel(x, w_weight, w_bias, proj_weight, proj_bias):
    out, _ = _run(x, w_weight, w_bias, proj_weight, proj_bias, trace=False)
    return out


def kernel_with_time(x, w_weight, w_bias, proj_weight, proj_bias):
    return _run(x, w_weight, w_bias, proj_weight, proj_bias, trace=True)


# revision 41
# speedup vs baseline: 1.1903x; 1.0164x over previous
"""Fused attention kernel for Trainium2, 8 NeuronCores.

Problem: B=4, T=2048, C=1024, nh=16, hs=64, fused QKV (chunk order k,q,v),
softmax attention, then (faithful reference bug) reshape (B,nh,T,hs)->(B,T,C)
directly before the output projection.

Key structural fact: with the buggy reshape, head h's attention output
occupies exactly rows [h*128, (h+1)*128) of the reshaped (T, C) matrix
(row tau = h*128 + t//16, col = (t%16)*64 + d). So everything after the
QKV projection is fully independent per (batch, head) pair; the output
projection needs no cross-head reduction.

Sharding: 8 cores = 4 batches x 2 head-groups (8 heads each). Each core
computes its batch's QKV slice and its 8 heads end-to-end. No collectives.

v2: single fused pipeline. The whole kernel is one stream of 256
attention iterations (hp, ic, j) with the QKV projection folded in as
prefix (K0,Q0,V) + per-block filler bursts, so the tensor engine never
drains and the scalar engine (exp) starts ~25us in. Scores operands in
bf16 (fp32 rhs streams ~1.5x slower). A fraction of the exp tiles run
on the DVE via a Schraudolph int16 bit-trick (exp error is a ~2% sawtooth
whose constant factor cancels in softmax normalization), keeping the
scalar engine below the tensor engine's per-iteration cost.
"""

import math
import sys

import numpy as np

sys.path.insert(0, "/opt/trn_rl_repo")

import ml_dtypes  # noqa: E402

B, T, C = 4, 2048, 1024
NH, HS = 16, 64
NCORES = 8
HPC = 8  # heads per core

_CACHE = {}

# Schraudolph fast-exp constants (bf16 bit pattern via int16):
# exp(0.125*x) ~= bitcast_bf16(int16(x * (0.125*128/ln2) + B)). B is shifted
# below 127*128=16256 to zero the mean of the sawtooth approximation error:
# a nonzero mean is a systematic softmax-weight bias for the key-blocks that
# take this path (the constant factor only cancels when ALL blocks share it).
EXP_A = 0.125 * 128.0 / math.log(2.0)
EXP_B = 16249.8


def _build():
    from contextlib import ExitStack

    import concourse.bass as bass  # noqa: F401
    import concourse.mybir as mybir
    from concourse import bacc, tile

    F32 = mybir.dt.float32
    BF16 = mybir.dt.bfloat16
    I16 = mybir.dt.int16
    ADD = mybir.AluOpType.add
    MULT = mybir.AluOpType.mult
    EXP = mybir.ActivationFunctionType.Exp
    IDENT = mybir.ActivationFunctionType.Identity

    nc = bacc.Bacc()
    # DRAM inputs (host-prepacked for contiguous, full-rate DMA rows)
    xq = nc.dram_tensor("xq", [8, 128, 2048], BF16, kind="ExternalInput")
    wkq = nc.dram_tensor("wkq", [8, 128, 1024], BF16, kind="ExternalInput")
    wvd = nc.dram_tensor("wvd", [128, 4096], BF16, kind="ExternalInput")
    bkq = nc.dram_tensor("bkq", [128, 8], F32, kind="ExternalInput")
    bv = nc.dram_tensor("bv", [128, 512], F32, kind="ExternalInput")
    wp = nc.dram_tensor("wp", [64, 16, 1024], BF16, kind="ExternalInput")
    pb = nc.dram_tensor("pb", [128, 1024], F32, kind="ExternalInput")
    vones = nc.dram_tensor("vones", [128, 1], BF16, kind="ExternalInput")
    onesr = nc.dram_tensor("onesr", [1, 64], F32, kind="ExternalInput")
    y = nc.dram_tensor("y", [128, 8, 1024], F32, kind="ExternalOutput")

    with tile.TileContext(nc) as tc, ExitStack() as ctx:
        persist = ctx.enter_context(tc.tile_pool(name="persist", bufs=1))
        utp = ctx.enter_context(tc.tile_pool(name="utp", bufs=10))
        usp = ctx.enter_context(tc.tile_pool(name="usp", bufs=2))
        nrm = ctx.enter_context(tc.tile_pool(name="nrm", bufs=2))
        yps = ctx.enter_context(tc.tile_pool(name="ysb", bufs=2))
        spx = ctx.enter_context(tc.tile_pool(name="spool", bufs=2, space="PSUM"))
        ypx = ctx.enter_context(tc.tile_pool(name="ypool", bufs=2, space="PSUM"))
        opx = ctx.enter_context(tc.tile_pool(name="opool", bufs=1, space="PSUM"))
        rpx = ctx.enter_context(tc.tile_pool(name="rpool", bufs=1, space="PSUM"))
        dpool = ctx.enter_context(tc.tile_pool(name="dpool", bufs=2, space="DRAM"))

        # ---- persistent tiles + priority-ordered input DMAs ----
        wkq_sb = [persist.tile([128, 1024], BF16, tag=f"wkq{mt}", name=f"wkq{mt}")
                  for mt in range(8)]
        xts = persist.tile([128, 8, 2048], BF16, tag="xts")
        wv_sb = persist.tile([128, 8, 512], BF16, tag="wv")
        bkq_sb = persist.tile([128, 8], F32, tag="bkq")
        bv_sb = persist.tile([128, 512], F32, tag="bv")
        vones_sb = persist.tile([128, 1], BF16, tag="vones")
        wp_sb = persist.tile([128, 16, 1024], BF16, tag="wp")
        pb_sb = persist.tile([128, 1024], F32, tag="pb")
        vbuf = persist.tile([128, 16, HPC, 64], BF16, tag="vbuf")
        qk = [persist.tile([128, 2048], BF16, tag=f"qk{mt}", name=f"qk{mt}")
              for mt in range(8)]
        ots = [persist.tile([128, 2048], BF16, tag=f"ot{hp}", name=f"ot{hp}")
               for hp in range(4)]

        # first-needed first: K0 weights, x half-quarters, Q0/K1/Q1 weights,
        # V weights, remaining KQ weights, then attention-phase constants.
        nc.sync.dma_start(wkq_sb[0], wkq[0])
        nc.sync.dma_start(bkq_sb, bkq[:])
        nc.sync.dma_start(
            xts[:, :, 0:256], xq[0].rearrange("p (a b) -> p a b", b=256))
        nc.sync.dma_start(
            xts[:, :, 256:512], xq[1].rearrange("p (a b) -> p a b", b=256))
        nc.sync.dma_start(wkq_sb[4], wkq[4])
        for q in range(2, 8):
            nc.sync.dma_start(
                xts[:, :, q * 256:(q + 1) * 256],
                xq[q].rearrange("p (a b) -> p a b", b=256))
        nc.sync.dma_start(wkq_sb[1], wkq[1])
        nc.sync.dma_start(wkq_sb[5], wkq[5])
        nc.sync.dma_start(wv_sb, wvd.rearrange("p (a b) -> p a b", b=512))
        nc.sync.dma_start(bv_sb, bv[:])
        for mt in (2, 6, 3, 7):
            nc.sync.dma_start(wkq_sb[mt], wkq[mt])
        nc.sync.dma_start(vones_sb, vones[:])
        onesr_sb = persist.tile([1, 64], F32, tag="onesr")
        nc.sync.dma_start(onesr_sb, onesr[:])
        nc.sync.dma_start(wp_sb[0:64], wp[:])
        nc.sync.dma_start(wp_sb[64:128], wp[:])
        nc.sync.dma_start(pb_sb, pb[:])

        # ---------------- emission helpers ----------------
        ITEMS = [(hp, ic, j) for hp in range(4) for ic in range(4)
                 for j in range(16)]
        DVE_J = {3, 7, 11, 15}
        DVE_J3 = DVE_J
        uts = {}
        usums = {}

        def kq_group(mt, ic2, chunk=512):
            ps = spx.tile([128, 1024], F32, tag="sp", name=f"kq{mt}_{ic2}")
            for half in range(1024 // chunk):
                t0 = ic2 * 1024 + half * chunk
                for ct in range(8):
                    nc.tensor.matmul(
                        ps[:, half * chunk:(half + 1) * chunk],
                        wkq_sb[mt][:, ct * 128:(ct + 1) * 128],
                        xts[:, ct, t0:t0 + chunk],
                        start=(ct == 0), stop=(ct == 7))
            # Identity shares the exp_and_others act table: no table thrash
            nc.scalar.activation(
                qk[mt][:, ic2 * 1024:(ic2 + 1) * 1024], ps, IDENT,
                bias=bkq_sb[:, mt:mt + 1])

        def v_group(tt):
            ps = ypx.tile([128, 512], F32, tag="yp", name=f"v{tt}")
            for ct in range(8):
                nc.tensor.matmul(
                    ps, xts[:, ct, tt * 128:(tt + 1) * 128], wv_sb[:, ct, :],
                    start=(ct == 0), stop=(ct == 7))
            nc.vector.tensor_tensor(
                vbuf[:, tt, :, :],
                ps.rearrange("p (h d) -> p h d", d=64),
                bv_sb.rearrange("p (h d) -> p h d", d=64), ADD)

        def emit_score(idx):
            hp, ic, j = ITEMS[idx]
            kt = qk[hp]
            qt = qk[4 + hp]
            jsl = slice(j * 128, (j + 1) * 128)
            isl = slice(ic * 512, (ic + 1) * 512)
            sp = spx.tile([128, 1024], F32, tag="sp", name="sp")
            nc.tensor.matmul(sp[:, 0:512], kt[0:64, jsl], qt[0:64, isl],
                             start=True, stop=True)
            nc.tensor.matmul(sp[:, 512:1024], kt[64:128, jsl],
                             qt[64:128, isl], start=True, stop=True)
            ut = utp.tile([128, 1024], BF16, tag="ut", name="ut")
            if j in (DVE_J3 if hp == 3 else DVE_J):
                nc.vector.tensor_scalar(
                    ut.bitcast(I16), sp, EXP_A, EXP_B, MULT, ADD)
            else:
                nc.scalar.activation(ut, sp, EXP, scale=0.125)
            uts[idx] = ut
            # pairwise-tree partial sums of exp tiles on the DVE: one
            # denominator matmul pair per 4 key-tiles instead of per 1.
            if j % 4 == 1:
                s1 = usp.tile([128, 1024], BF16, tag="us1", name="us1")
                nc.vector.tensor_tensor(s1, uts[idx - 1], ut, ADD)
                usums[(idx - 1) // 4] = s1
            elif j % 4 == 3:
                s2 = usp.tile([128, 1024], BF16, tag="us1", name="us2")
                nc.vector.tensor_tensor(s2, uts[idx - 1], ut, ADD)
                s12 = usp.tile([128, 1024], BF16, tag="us12", name="us12")
                nc.vector.tensor_tensor(s12, usums[(idx - 3) // 4], s2, ADD)
                usums[(idx - 3) // 4] = s12

        av_state = {}

        def emit_av(idx):
            hp, ic, j = ITEMS[idx]
            hA, hB = 2 * hp, 2 * hp + 1
            if j == 0:
                av_state["op"] = opx.tile([128, 512], F32, tag="op", name="op")
                av_state["rs"] = rpx.tile([33, 512], F32, tag="rs", name="rs")
            optile = av_state["op"]
            ut = uts.pop(idx)
            nc.tensor.matmul(optile[0:64, :], vbuf[:, j, hA, :], ut[:, 0:512],
                             start=(j == 0), stop=(j == 15))
            nc.tensor.matmul(optile[64:128, :], vbuf[:, j, hB, :],
                             ut[:, 512:1024],
                             start=(j == 0), stop=(j == 15),
                             tile_position=(0, 64))

        def emit_ones_quad(idx):
            # idx = last item of a quad; sums exp over key-tiles 4qd..4qd+3
            qd = (idx % 16) // 4
            rsps = av_state["rs"]
            us = usums.pop(idx // 4)
            nc.tensor.matmul(rsps[0:1, :], vones_sb, us[:, 0:512],
                             start=(qd == 0), stop=(qd == 3))
            nc.tensor.matmul(rsps[32:33, :], vones_sb, us[:, 512:1024],
                             start=(qd == 0), stop=(qd == 3),
                             tile_position=(0, 32))

        norm_state = {}

        def norm_stage1(hp, ic):
            rsps = av_state["rs"]
            rsb = nrm.tile([33, 512], F32, tag="rsb", name="rsb")
            nc.vector.tensor_copy(rsb, rsps)
            osb = nrm.tile([128, 512], F32, tag="osb", name="osb")
            nc.vector.tensor_copy(osb, av_state["op"])
            scr1 = dpool.tile([1024], F32, tag="scr1", name="scr1")
            nc.sync.dma_start(
                scr1.rearrange("(r f) -> r f", r=2), rsb[0:33:32, :])
            rst = nrm.tile([128, 8], F32, tag="rst", name="rst")
            nc.sync.dma_start(rst, scr1.rearrange("(p f) -> p f", f=8))
            norm_state[(hp, ic)] = (osb, rst)

        def norm_tail_fast(hp, ic):
            # last block: skip the DRAM broadcast round-trip; reciprocal on
            # the row copies, partition-broadcast via a PE outer product into
            # now-free PSUM, multiply from there.
            optile, rsps = av_state["op"], av_state["rs"]
            rsb = nrm.tile([33, 512], F32, tag="rsb", name="rsb")
            nc.vector.tensor_copy(rsb, rsps)
            osb = nrm.tile([128, 512], F32, tag="osb", name="osb")
            nc.vector.tensor_copy(osb, optile)
            rsbB = persist.tile([1, 512], F32, tag="rsbB", name="rsbB")
            nc.sync.dma_start(rsbB, rsb[32:33, :])
            nc.vector.reciprocal(rsb[0:1, :], rsb[0:1, :])
            nc.vector.reciprocal(rsbB, rsbB)
            bc = spx.tile([128, 1024], F32, tag="sp", name="bc")
            nc.tensor.matmul(bc[0:64, 0:512], onesr_sb, rsb[0:1, :],
                             start=True, stop=True)
            nc.tensor.matmul(bc[64:128, 0:512], onesr_sb, rsbB,
                             start=True, stop=True, tile_position=(0, 64))
            nc.vector.tensor_tensor(
                ots[hp][:, ic * 512:(ic + 1) * 512], osb, bc[:, 0:512], MULT)

        def norm_stage2(hp, ic):
            osb, rst = norm_state[(hp, ic)]
            nc.vector.reciprocal(rst, rst)
            scr2 = dpool.tile([1024], F32, tag="scr2", name="scr2")
            nc.sync.dma_start(scr2.rearrange("(p f) -> p f", f=8), rst)
            bcsb = nrm.tile([128, 512], F32, tag="bcsb", name="bcsb")
            nc.sync.dma_start(
                bcsb[0:64, :], scr2[None, 0:512].to_broadcast((64, 512)))
            nc.sync.dma_start(
                bcsb[64:128, :], scr2[None, 512:1024].to_broadcast((64, 512)))
            norm_state[(hp, ic)] = (osb, bcsb)

        def norm_stage3(hp, ic):
            osb, bcsb = norm_state.pop((hp, ic))
            nc.vector.tensor_tensor(
                ots[hp][:, ic * 512:(ic + 1) * 512], osb, bcsb, MULT)

        proj_state = {}

        def proj_step(hp, q2, u):
            if "ypA" not in proj_state:
                proj_state["ypA"] = ypx.tile([128, 512], F32, tag="yp",
                                             name="ypA")
                proj_state["ypB"] = ypx.tile([128, 512], F32, tag="yp",
                                             name="ypB")
            ypA, ypB = proj_state["ypA"], proj_state["ypB"]
            otr = ots[hp].rearrange("d (t u) -> d u t", u=16)
            csl = slice(q2 * 512, (q2 + 1) * 512)
            nc.tensor.matmul(ypA, otr[0:64, u, :], wp_sb[0:64, u, csl],
                             start=(u == 0), stop=(u == 15))
            nc.tensor.matmul(ypB, otr[64:128, u, :], wp_sb[64:128, u, csl],
                             start=(u == 0), stop=(u == 15))
            if u == 15:
                for h, yp in ((2 * hp, ypA), (2 * hp + 1, ypB)):
                    ysb = yps.tile([128, 512], F32, tag="ysb", name="ysb")
                    nc.vector.tensor_tensor(ysb, yp, pb_sb[:, csl], ADD)
                    nc.sync.dma_start(y[:, h, csl], ysb)
                proj_state.clear()

        # ---------------- prefix: K0,Q0,K1,Q1, V, first scores -----------
        LOOK = 8
        nscore = 0

        def next_score():
            nonlocal nscore
            emit_score(nscore)
            nscore += 1

        kq_group(0, 0, chunk=256)
        kq_group(4, 0)
        kq_group(0, 1)
        kq_group(4, 1)
        kq_group(1, 0)
        next_score()
        kq_group(5, 0)
        next_score()
        kq_group(1, 1)
        next_score()
        kq_group(5, 1)
        next_score()
        for tt in range(16):
            v_group(tt)
            if tt in (5, 8, 11, 14) and nscore < LOOK:
                next_score()

        # ------------- main loop: 16 blocks x 8 groups of 2 iters --------
        FILLER = [(2, 0), (2, 1), (6, 0), (6, 1), (3, 0), (3, 1),
                  (7, 0), (7, 1)]
        BLOCKS = [(hp, ic) for hp in range(4) for ic in range(4)]
        proj_q = []
        prev_block = None

        for bi, (hp, ic) in enumerate(BLOCKS):
            if bi < 8:
                kq_group(*FILLER[bi])
            for g in range(8):
                w = 16 * bi + 2 * g
                if proj_q:
                    proj_step(*proj_q.pop(0))
                    if len(proj_q) > 8:
                        proj_step(*proj_q.pop(0))
                if nscore < 256:
                    next_score()
                if nscore < 256:
                    next_score()
                emit_av(w)
                emit_av(w + 1)
                if g % 2 == 1:
                    emit_ones_quad(w + 1)
                if prev_block is not None:
                    if g == 2:
                        norm_stage2(*prev_block)
                    elif g == 4:
                        norm_stage3(*prev_block)
                    elif g == 5 and prev_block[1] == 3:
                        php = prev_block[0]
                        proj_q.extend([(php, q2, u) for q2 in range(2)
                                       for u in range(16)])
            if bi < 15:
                norm_stage1(hp, ic)
            else:
                norm_tail_fast(hp, ic)
            prev_block = (hp, ic)

        # ---------------- tail ----------------
        proj_q.extend([(3, q2, u) for q2 in range(2) for u in range(16)])
        while proj_q:
            proj_step(*proj_q.pop(0))

    nc.compile()
    return nc


def _in_maps(x, w_weight, w_bias, proj_weight, proj_bias):
    x = np.ascontiguousarray(x, np.float32)
    w_weight = np.ascontiguousarray(w_weight, np.float32)
    w_bias = np.ascontiguousarray(w_bias, np.float32)
    proj_weight = np.ascontiguousarray(proj_weight, np.float32)
    proj_bias = np.ascontiguousarray(proj_bias, np.float32)

    wpT = np.ascontiguousarray(
        proj_weight.T.reshape(16, 64, 1024).transpose(1, 0, 2).astype(ml_dtypes.bfloat16))
    pbr = np.ascontiguousarray(np.tile(proj_bias[None], (128, 1)))
    vones = np.ones((128, 1), dtype=ml_dtypes.bfloat16)

    maps = []
    for c in range(NCORES):
        b = c // 2
        h0 = (c % 2) * HPC
        # x[b] -> [part(c%128), ct, t] in eighth-of-t chunks
        xT = x[b].T.reshape(8, 128, 2048).transpose(1, 0, 2)  # [p, ct, t]
        xqc = np.ascontiguousarray(
            xT.reshape(128, 8, 8, 256).transpose(2, 0, 1, 3)
            .reshape(8, 128, 2048).astype(ml_dtypes.bfloat16))
        # K tiles (mt 0-3), Q tiles (mt 4-7): [mt][p, ct*128 + m]
        wk = w_weight[h0 * 64: h0 * 64 + 512]
        wq = w_weight[1024 + h0 * 64: 1024 + h0 * 64 + 512]
        wkqc = np.zeros((8, 128, 1024), np.float32)
        for mt in range(4):
            for src, off in ((wk, 0), (wq, 4)):
                rows = src[mt * 128:(mt + 1) * 128]  # [128m, 1024c]
                wkqc[mt + off] = rows.T.reshape(8, 128, 128).transpose(
                    1, 0, 2).reshape(128, 1024)
        wkqc = np.ascontiguousarray(wkqc.astype(ml_dtypes.bfloat16))
        wv = w_weight[2048 + h0 * 64: 2048 + h0 * 64 + 512]  # [512m, 1024c]
        wvc = np.ascontiguousarray(
            wv.T.reshape(8, 128, 512).transpose(1, 0, 2)
            .reshape(128, 4096).astype(ml_dtypes.bfloat16))
        bk = w_bias[h0 * 64: h0 * 64 + 512]
        bq = w_bias[1024 + h0 * 64: 1024 + h0 * 64 + 512]
        bvc = w_bias[2048 + h0 * 64: 2048 + h0 * 64 + 512]
        bkqc = np.ascontiguousarray(
            np.concatenate([bk.reshape(4, 128).T, bq.reshape(4, 128).T], axis=1))
        bvr = np.ascontiguousarray(np.tile(bvc[None], (128, 1)))
        maps.append({
            "xq": xqc, "wkq": wkqc, "wvd": wvc, "bkq": bkqc, "bv": bvr,
            "wp": wpT, "pb": pbr, "vones": vones,
            "onesr": np.ones((1, 64), np.float32),
        })
    return maps


def _install_ntff_hook():
    """Register the axon NTFF profiling hook (missing antenv.axon_hooks shim)."""
    import contextlib
    import ctypes
    import types

    if "antenv.axon_hooks" in sys.modules:
        return
    import antenv
    so_path = "/opt/axon/libaxon_pjrt.so"
    try:
        lib = ctypes.CDLL(so_path)
    except OSError:
        return
    if not hasattr(lib, "axon_start_nrt_profile"):
        return
    lib.axon_start_nrt_profile.argtypes = [ctypes.POINTER(ctypes.c_int64),
                                           ctypes.c_size_t]
    lib.axon_start_nrt_profile.restype = ctypes.c_int64
    lib.axon_stop_nrt_profile.argtypes = [ctypes.c_char_p]
    lib.axon_stop_nrt_profile.restype = ctypes.c_int64

    @contextlib.contextmanager
    def _hook(output_dir, device_ids):
        import jax
        jax.devices()
        if device_ids:
            ids = (ctypes.c_int64 * len(device_ids))(*device_ids)
            rc = lib.axon_start_nrt_profile(ids, len(device_ids))
        else:
            rc = lib.axon_start_nrt_profile(None, 0)
        if rc != 0:
            raise RuntimeError(f"axon_start_nrt_profile rc={rc}")
        try:
            yield
        finally:
            n = lib.axon_stop_nrt_profile(str(output_dir).encode())
            print(f"profile: {n} file(s) written to {output_dir}", file=sys.stderr)

    mod = types.ModuleType("antenv.axon_hooks")
    mod.get_axon_ntff_profile_hook = lambda: _hook
    mod.set_axon_ntff_profile_hook = lambda h: None
    sys.modules["antenv.axon_hooks"] = mod
    antenv.axon_hooks = mod


def _run(x, w_weight, w_bias, proj_weight, proj_bias, trace=False):
    from concourse.bass_utils import run_bass_kernel_spmd

    if trace:
        _install_ntff_hook()

    if "nc" not in _CACHE:
        _CACHE["nc"] = _build()
    nc = _CACHE["nc"]
    maps = _in_maps(x, w_weight, w_bias, proj_weight, proj_bias)
    res = run_bass_kernel_spmd(nc, maps, core_ids=list(range(NCORES)), trace=trace)
    out = np.zeros((B, T, C), np.float32)
    for c in range(NCORES):
        yc = res.results[c]["y"]  # [128, 8, 1024]
        b = c // 2
        h0 = (c % 2) * HPC
        for j in range(HPC):
            out[b, (h0 + j) * 128:(h0 + j + 1) * 128, :] = yc[:, j, :]
    return out, res.exec_time_ns


def kernel(x, w_weight, w_bias, proj_weight, proj_bias):
    out, _ = _run(x, w_weight, w_bias, proj_weight, proj_bias, trace=False)
    return out


def kernel_with_time(x, w_weight, w_bias, proj_weight, proj_bias):
    return _run(x, w_weight, w_bias, proj_weight, proj_bias, trace=True)
